# revision 7
# baseline (speedup 1.0000x reference)
"""Confidence-weighted mutual cross-attention on 8 Trainium2 NeuronCores.

Reference (per batch b of 8):
    q = (lidar @ Wq.T + bq) * lidar_conf        [N=2048, D=512]
    k = camera @ Wk.T + bk                      [M=2048, D=512]
    v = camera @ Wv.T + bv                      [M=2048, D=512]
    out = softmax(q @ k.T, axis=-1) @ v         [N, D]
(camera_confidence is unused by the reference.)

Sharding: data-parallel over batch — one batch element per NeuronCore,
fully fused on-chip.

Algebraic restructure (saves the K projection + all input transposes):
    q·k = (lidar Wq^T + bq)·(cam Wk^T + bk)
        = lidar (Wq^T Wk) cam^T + (bq Wk)·cam^T + [per-row const] + const
The per-row terms are softmax-invariant and drop. With C = Wq^T Wk and
q~ = conf * (lidar C + bq Wk):
    scores_eff = q~ @ cam^T        (cam^T comes PRE-TRANSPOSED from host)
so "K" is the raw transposed camera features — no K projection. The
row-wise conf scaling stays folded into q~ (softmax is NOT invariant
to it, matching the reference).

Per-core dataflow (f32r matmuls = single-pass fp32 on the PE):
  phase A: DMA raw Wq/Wk -> C on the PE (16 fp32 matmuls); lidar^T
           (host-transposed) arrives per column-slab, DVE-cast to f32r,
           -> Q~^T with bias+conf folded in the PSUM->SBUF pass; cam^T
           arrives, ACT-cast to f32r (doubles as S-rhs and V-proj
           lhsT) -> V = cam Wv^T (+bv) in bf16. Zero PE transposes.
  phase B: per 128-row q-tile: S = Q~^T.T @ cam^T in PSUM [128, 2048],
           mb-outer so a DVE partial row-max chases each 512-slab and
           exp starts right after the last slab; exp + row-sum in one
           ACT pass emitting bf16 P; P^T via 16 bf16 PE transposes
           (batched 4 per PSUM bank, ACT copies out); O = P^T.T @ V
           (bf16, fp32 accum); normalize by row-sum on DVE; DMA out.
           Software-pipelined: PV(t-1) fills softmax(t) latency.
"""

import contextlib

import numpy as np

import concourse.bass as bass
import concourse.mybir as mybir
import concourse.tile as tile
from concourse import bacc
from concourse.bass_utils import run_bass_kernel_spmd

F32 = mybir.dt.float32
F32R = mybir.dt.float32r
BF16 = mybir.dt.bfloat16
AX = mybir.AxisListType
OP = mybir.AluOpType
AF = mybir.ActivationFunctionType

B, N, M, D = 8, 2048, 2048, 512
DC = D // 128   # 128-chunks of the model dim
NT = N // 128   # q tiles
MT = M // 128   # kv tiles
NB = N // 512   # 512-wide column slabs
MB = M // 512


def _bcast(ap_1d: bass.AP, parts: int = 128) -> bass.AP:
    """1-D DRAM vector AP -> [parts, L] AP replicated over partitions."""
    return bass.AP(
        tensor=ap_1d.tensor,
        offset=ap_1d.offset,
        ap=[[0, parts]] + [list(x) for x in ap_1d.ap],
    )


def build():
    nc = bacc.Bacc(None)

    # Host ships lidar^T/cam^T ([feature, token]) and Wv^T; Wq/Wk raw.
    lidar_t = nc.declare_dram_parameter("lidar_t", [D, N], F32, isOutput=False)
    cam_t = nc.declare_dram_parameter("cam_t", [D, M], F32, isOutput=False)
    lconf = nc.declare_dram_parameter("lconf", [N, 1], F32, isOutput=False)
    wq = nc.declare_dram_parameter("wq", [D, D], F32, isOutput=False)
    wk = nc.declare_dram_parameter("wk", [D, D], F32, isOutput=False)
    wvt = nc.declare_dram_parameter("wvt", [D, D], F32, isOutput=False)
    bqk = nc.declare_dram_parameter("bqk", [D], F32, isOutput=False)  # bq @ Wk
    bv = nc.declare_dram_parameter("bv", [D], F32, isOutput=False)
    out = nc.declare_dram_parameter("out", [N, D], F32, isOutput=True)

    with tile.TileContext(nc) as tc, contextlib.ExitStack() as ctx:
        persist = ctx.enter_context(tc.tile_pool(name="persist", bufs=1))
        ident = persist.tile([128, 128], BF16)
        from concourse.masks import make_identity

        make_identity(nc, ident[:])

        qt = persist.tile([128, DC, N], F32R)      # Q~^T: [e%128, e//128, n]
        cam_r = persist.tile([128, DC, M], F32R)   # cam^T: [e%128, e//128, m]
        v_sb = persist.tile([128, MT, D], BF16)    # V:  [m%128, m//128, d]

        with tc.tile_pool(name="phA", bufs=1) as pa, \
             tc.tile_pool(name="stg", bufs=2) as stg, \
             tc.tile_pool(name="psA", bufs=2, space="PSUM") as psA:
            # ---- issue ALL input DMAs up front, split across both hwdge
            # queues so they stream in parallel (~2x 190GB/s):
            #   sync:   Wq, Wk, lidar^T slabs   (gates C then Q~)
            #   scalar: cam^T slabs, Wv^T       (gates V and phase B)
            #   gpsimd: biases + confidence     (small)
            wq_sb = pa.tile([128, DC, D], F32)
            wk_sb = pa.tile([128, DC, D], F32)
            nc.sync.dma_start(out=wq_sb[:], in_=wq[:, :].rearrange("(c p) g -> p c g", p=128))
            nc.sync.dma_start(out=wk_sb[:], in_=wk[:, :].rearrange("(c p) e -> p c e", p=128))

            cam_st = pa.tile([128, DC, M], F32)
            wv_st = pa.tile([128, DC, D], F32)
            cam_view = cam_t[:, :].rearrange("(c p) m -> p c m", p=128)
            for mb in range(MB):
                sl = slice(mb * 512, (mb + 1) * 512)
                nc.scalar.dma_start(out=cam_st[:, :, sl], in_=cam_view[:, :, sl])
            nc.scalar.dma_start(out=wv_st[:], in_=wvt[:, :].rearrange("(c p) d -> p c d", p=128))

            bq_t = pa.tile([128, DC], F32)
            bv_bc = pa.tile([128, D], F32)
            conf_bc = pa.tile([128, N], F32)
            nc.gpsimd.dma_start(out=bq_t[:], in_=bqk[:].rearrange("(c p) -> p c", p=128))
            nc.gpsimd.dma_start(out=bv_bc[:], in_=_bcast(bv[:]))
            nc.gpsimd.dma_start(out=conf_bc[:], in_=_bcast(lconf[:, 0]))

            # --- C = Wq^T Wk  [g, e] -> c_sb [g%128, g//128, e]. Wq/Wk
            # stay plain f32 (LOW_HIGH matmuls) — only 16 of them, and it
            # keeps the DMA->C critical path cast-free.
            c_sb = pa.tile([128, DC, D], F32R)
            for gc in range(DC):
                pc_ = psA.tile([128, D], F32, name=f"pc_{gc}", tag="proj")
                for fc in range(DC):
                    nc.tensor.matmul(
                        pc_[:],
                        wq_sb[:, fc, gc * 128:(gc + 1) * 128],
                        wk_sb[:, fc, :],
                        start=(fc == 0),
                        stop=(fc == DC - 1),
                    )
                nc.scalar.copy(c_sb[:, gc, :], pc_[:])

            # --- lidar^T per column slab -> Q~^T = (lidar C)^T + bias, *conf
            li_view = lidar_t[:, :].rearrange("(c p) n -> p c n", p=128)
            for nb in range(NB):
                sl = slice(nb * 512, (nb + 1) * 512)
                li_st = stg.tile([128, DC, 512], F32, name=f"li_st_{nb}", tag="li_st")
                li_r = stg.tile([128, DC, 512], F32R, name=f"li_r_{nb}", tag="li_r")
                nc.sync.dma_start(out=li_st[:], in_=li_view[:, :, sl])
                nc.vector.tensor_copy(li_r[:], li_st[:])  # f32r rounding
                for ec in range(DC):
                    pq = psA.tile([128, 512], F32, name=f"pq_{ec}_{nb}", tag="proj")
                    for gc in range(DC):
                        nc.tensor.matmul(
                            pq[:],
                            c_sb[:, gc, ec * 128:(ec + 1) * 128],
                            li_r[:, gc, :],
                            start=(gc == 0),
                            stop=(gc == DC - 1),
                        )
                    # q~^T = (proj + (bq Wk)[e]) * conf[n]
                    nc.vector.scalar_tensor_tensor(
                        out=qt[:, ec, sl],
                        in0=pq[:],
                        scalar=bq_t[:, ec:ec + 1],
                        in1=conf_bc[:, sl],
                        op0=OP.add,
                        op1=OP.mult,
                    )

            # --- camera^T casts (ACT) -> V = cam Wv^T + bv (bf16)
            wv_r = pa.tile([128, DC, D], F32R)
            nc.scalar.copy(wv_r[:], wv_st[:])
            for mb in range(MB):
                sl = slice(mb * 512, (mb + 1) * 512)
                nc.scalar.copy(cam_r[:, :, sl], cam_st[:, :, sl])
                for mt in range(4 * mb, 4 * mb + 4):
                    pv = psA.tile([128, D], F32, name=f"pv_{mt}", tag="proj")
                    for ec in range(DC):
                        nc.tensor.matmul(
                            pv[:],
                            cam_r[:, ec, mt * 128:(mt + 1) * 128],
                            wv_r[:, ec, :],
                            start=(ec == 0),
                            stop=(ec == DC - 1),
                        )
                    nc.vector.tensor_tensor(
                        out=v_sb[:, mt, :], in0=pv[:], in1=bv_bc[:], op=OP.add
                    )

        # ---------------- phase B: attention ----------------
        with tc.tile_pool(name="pexp", bufs=4) as pexp, \
             tc.tile_pool(name="small", bufs=6) as small, \
             tc.tile_pool(name="psS", bufs=1, space="PSUM") as psS, \
             tc.tile_pool(name="psPT", bufs=2, space="PSUM") as psPT, \
             tc.tile_pool(name="psO", bufs=2, space="PSUM") as psO:

            def emit_scores_softmax(t):
                """S(t) (mb-outer; DVE partial row-max chases each slab) ->
                exp(t); returns (p_bf, sums)."""
                s_ps = psS.tile([128, M], F32, name=f"s_{t}", tag="S")
                pm = small.tile([128, MB], F32, name=f"pm_{t}", tag="pmax")
                for mb in range(MB):
                    sl = slice(mb * 512, (mb + 1) * 512)
                    for ec in range(DC):
                        nc.tensor.matmul(
                            s_ps[:, sl],
                            qt[:, ec, t * 128:(t + 1) * 128],
                            cam_r[:, ec, sl],
                            start=(ec == 0),
                            stop=(ec == DC - 1),
                        )
                    nc.vector.tensor_reduce(
                        out=pm[:, mb:mb + 1], in_=s_ps[:, sl], axis=AX.X, op=OP.max
                    )
                negmax = small.tile([128, 1], F32, name=f"nm_{t}", tag="negmax")
                nc.vector.tensor_reduce(
                    out=negmax[:], in_=pm[:], axis=AX.X, op=OP.max, negate=True
                )
                p_bf = pexp.tile([128, M], BF16, name=f"p_{t}", tag="P")
                sums = small.tile([128, 1], F32, name=f"sum_{t}", tag="sums")
                nc.scalar.activation(
                    out=p_bf[:],
                    in_=s_ps[:],
                    func=AF.Exp,
                    bias=negmax[:],
                    scale=1.0,
                    accum_out=sums[:],
                )
                return p_bf, sums

            def emit_pv(t, p_bf, sums):
                """P^T(t) on the PE (exp(t) is already done by now, so the
                PE never waits) -> PV(t) -> normalize -> DMA out."""
                ptr = pexp.tile([128, MT, 128], BF16, name=f"ptr_{t}", tag="PT")
                for g in range(MT // 4):
                    ptp = psPT.tile(
                        [128, 4, 128], BF16, name=f"ptp_{t}_{g}", tag="ptp"
                    )
                    for c in range(4):
                        j = g * 4 + c
                        nc.tensor.transpose(
                            ptp[:, c, :], p_bf[:, j * 128:(j + 1) * 128], ident[:]
                        )
                    nc.scalar.copy(ptr[:, g * 4:(g + 1) * 4, :], ptp[:])

                o_ps = psO.tile([128, D], F32, name=f"o_{t}", tag="O")
                for j in range(MT):
                    nc.tensor.matmul(
                        o_ps[:],
                        ptr[:, j, :],
                        v_sb[:, j, :],
                        start=(j == 0),
                        stop=(j == MT - 1),
                    )
                recip = small.tile([128, 1], F32, name=f"rc_{t}", tag="recip")
                nc.vector.reciprocal(recip[:], sums[:])
                o_sb = pexp.tile([128, D], F32, name=f"o_sb_{t}", tag="Osb")
                nc.vector.tensor_scalar_mul(out=o_sb[:], in0=o_ps[:], scalar1=recip[:])
                nc.gpsimd.dma_start(out=out[t * 128:(t + 1) * 128, :], in_=o_sb[:])

            # Software pipeline: PV(t-1) fills the PE while softmax(t)
            # runs on ACT/DVE.
            pending = None
            for t in range(NT):
                cur = emit_scores_softmax(t)
                if pending is not None:
                    emit_pv(t - 1, *pending)
                pending = cur
            emit_pv(NT - 1, *pending)

    nc.compile()
    return nc


_NC_CACHE = None


def make_in_maps(inputs) -> list[dict]:
    def f32(name):
        return np.ascontiguousarray(np.asarray(inputs[name]), dtype=np.float32)

    li, ca, lc = f32("lidar_features"), f32("camera_features"), f32("lidar_confidence")
    wq_, wk_ = f32("Wq"), f32("Wk")
    wvt_ = np.ascontiguousarray(f32("Wv").T)
    bqk_ = np.ascontiguousarray(f32("bq") @ wk_)  # (bq Wk)[e]
    bv_ = f32("bv")

    return [
        {
            "lidar_t": np.ascontiguousarray(li[b].T),
            "cam_t": np.ascontiguousarray(ca[b].T),
            "lconf": lc[b],
            "wq": wq_, "wk": wk_, "wvt": wvt_,
            "bqk": bqk_, "bv": bv_,
        }
        for b in range(B)
    ]


def kernel(**inputs) -> np.ndarray:
    global _NC_CACHE
    if _NC_CACHE is None:
        _NC_CACHE = build()
    nc = _NC_CACHE

    res = run_bass_kernel_spmd(nc, make_in_maps(inputs), list(range(B)))
    return np.stack([res.results[b]["out"] for b in range(B)]).astype(np.float32)


# revision 59
# speedup vs baseline: 1.3102x; 1.3102x over previous
"""Confidence-weighted mutual cross-attention on 8 Trainium2 NeuronCores.

Reference (per batch b of 8):
    q = (lidar @ Wq.T + bq) * lidar_conf        [N=2048, D=512]
    k = camera @ Wk.T + bk                      [M=2048, D=512]
    v = camera @ Wv.T + bv                      [M=2048, D=512]
    out = softmax(q @ k.T, axis=-1) @ v         [N, D]
(camera_confidence is unused by the reference.)

Sharding: data-parallel over batch — one batch element per NeuronCore,
fully fused on-chip.

Algebraic restructure (kills the K projection + all input transposes):
    q·k = (lidar Wq^T + bq)·(cam Wk^T + bk)
        = lidar (Wq^T Wk) cam^T + (bq Wk)·cam^T + [per-row const] + const
The per-row terms are softmax-invariant and drop. With C = Wq^T Wk and
q~ = conf * (lidar C + bq Wk):
    scores_eff = q~ @ cam^T        (cam^T comes PRE-TRANSPOSED from host)
so "K" is the raw transposed camera features — no K projection. The
row-wise conf scaling stays folded into q~ (softmax is NOT invariant to
it, matching the reference).

Schedule (single fused pipeline, PE is the pacer; ~208-209us HW
measured vs 241us for the classic Q/K/V-projection + PE-transpose
structure):
  prologue: C = Wq^T Wk chunk-chased behind Wq/Wk chunk DMAs
            alternating across both hwdge queues (f32r via DVE casts,
            C psum copies on DVE so ACT is free for cam casts); cam^T
            slabs split across BOTH hwdge queues in S(0)'s consumption
            order + ACT f32r casts; Wv^T f32r via gpsimd cast-DMA.
            Zero PE transposes of inputs. S(0) launches as soon as the
            last cam cast lands — the V projection is NOT on its
            critical path; V slots into the tile-0/1 softmax windows
            as pipeline cover instead (PV(0) waits for it).
  main loop over 16 q-tiles, with the Q~ projection of column slab nb
  (its lidar DMA prefetched one slab ahead) landing right before the
  4 tiles that read it:
    S(t) = Q~^T.T @ cam^T into two half-width PSUM tiles (tag bufs=2,
    so S(t+1)'s first half only waits exp(t)'s first half — the
    single-buffer S->exp serialization was the #1 steady-state
    bottleneck); slabs tapered 512/512/512/384/128 with a DVE partial
    row-max chasing each one so the final max is ~0.2us after the
    last matmul; exp + row-sum in two ACT passes emitting bf16 P;
    then (pipelined one tile back) P^T via 16 bf16 PE transposes
    (batched 8 per PSUM bank, DVE copies out so ACT's exp is never
    queued behind them) and O = P^T.T @ V (bf16, fp32 accum);
    normalize by row-sum on DVE; DMA out on the sync hwdge queue (the
    gpsimd software queue adds ~10us dispatch latency to the tail).
PSUM: C-accum and the S halves share one 4-bank tag; Q~/V/O psums
share one 2-bank tag (bufs=2); P^T staging 2 banks.
"""

import contextlib

import numpy as np

import concourse.bass as bass
import concourse.mybir as mybir
import concourse.tile as tile
from concourse import bacc
from concourse.bass_utils import run_bass_kernel_spmd

F32 = mybir.dt.float32
F32R = mybir.dt.float32r
BF16 = mybir.dt.bfloat16
AX = mybir.AxisListType
OP = mybir.AluOpType
AF = mybir.ActivationFunctionType

B, N, M, D = 8, 2048, 2048, 512
DC = D // 128   # 128-chunks of the model dim
NT = N // 128   # q tiles
MT = M // 128   # kv tiles
NB = N // 512   # 512-wide column slabs
MB = M // 512


def _bcast(ap_1d: bass.AP, parts: int = 128) -> bass.AP:
    """1-D DRAM vector AP -> [parts, L] AP replicated over partitions."""
    return bass.AP(
        tensor=ap_1d.tensor,
        offset=ap_1d.offset,
        ap=[[0, parts]] + [list(x) for x in ap_1d.ap],
    )


def build():
    nc = bacc.Bacc(None)

    # Host ships lidar^T/cam^T ([feature, token]) and Wv^T; Wq/Wk raw.
    lidar_t = nc.declare_dram_parameter("lidar_t", [D, N], F32, isOutput=False)
    cam_t = nc.declare_dram_parameter("cam_t", [D, M], F32, isOutput=False)
    lconf = nc.declare_dram_parameter("lconf", [N, 1], F32, isOutput=False)
    wq = nc.declare_dram_parameter("wq", [D, D], F32, isOutput=False)
    wk = nc.declare_dram_parameter("wk", [D, D], F32, isOutput=False)
    wvt = nc.declare_dram_parameter("wvt", [D, D], F32, isOutput=False)
    bqk = nc.declare_dram_parameter("bqk", [D], F32, isOutput=False)  # bq @ Wk
    bv = nc.declare_dram_parameter("bv", [D], F32, isOutput=False)
    out = nc.declare_dram_parameter("out", [N, D], F32, isOutput=True)

    with tile.TileContext(nc) as tc, contextlib.ExitStack() as ctx:
        persist = ctx.enter_context(tc.tile_pool(name="persist", bufs=1))
        ident = persist.tile([128, 128], BF16)
        from concourse.masks import make_identity

        make_identity(nc, ident[:])

        qt = persist.tile([128, DC, N], F32R)      # Q~^T: [e%128, e//128, n]
        cam_r = persist.tile([128, DC, M], F32R)   # cam^T: [e%128, e//128, m]
        v_sb = persist.tile([128, MT, D], BF16)    # V:  [m%128, m//128, d]

        pa = ctx.enter_context(tc.tile_pool(name="phA", bufs=1))
        stg = ctx.enter_context(tc.tile_pool(name="stg", bufs=2))
        # PSUM: "big" = C-accum / S scores (4 banks, serial reuse);
        # "proj" = Q~/V/O 512-wide psums (2 bufs); "ptp" = P^T staging.
        psB = ctx.enter_context(tc.tile_pool(name="psB", bufs=1, space="PSUM"))
        psP = ctx.enter_context(tc.tile_pool(name="psP", bufs=2, space="PSUM"))
        psT = ctx.enter_context(tc.tile_pool(name="psT", bufs=2, space="PSUM"))

        # ---- DMA issue, in consumption order per queue ----
        # sync(q_a): Wq chunks, lidar^T slabs (then outputs in phase B)
        # scalar(q_b): Wk chunks, then cam^T slabs (+ACT casts later)
        # gpsimd(sw): biases, confidence, Wv^T (cast-DMA -> f32r)
        bq_t = pa.tile([128, DC], F32)
        bv_bc = pa.tile([128, D], F32)
        conf_bc = pa.tile([128, N], F32)
        wv_r = pa.tile([128, DC, D], F32R)
        nc.gpsimd.dma_start(out=bq_t[:], in_=bqk[:].rearrange("(c p) -> p c", p=128))

        c_sb = pa.tile([128, DC, D], F32R)
        li_view = lidar_t[:, :].rearrange("(c p) n -> p c n", p=128)

        # ---- prologue: C (chunk-chased, f32r), then V (cam-cast-chased) ----
        with tc.tile_pool(name="wC", bufs=1) as wc:
            nc.gpsimd.dma_start(out=bv_bc[:], in_=_bcast(bv[:]))
            nc.gpsimd.dma_start(out=conf_bc[:], in_=_bcast(lconf[:, 0]))
            nc.gpsimd.dma_start(  # f32r cast in flight (software DGE)
                out=wv_r[:], in_=wvt[:, :].rearrange("(c p) d -> p c d", p=128)
            )

            wq_st = wc.tile([128, DC, D], F32)
            wk_st = wc.tile([128, DC, D], F32)
            for fc in range(DC):
                # alternate queues so chunk PAIR fc completes every ~1.4us
                qa = nc.sync if fc % 2 == 0 else nc.scalar
                qb = nc.scalar if fc % 2 == 0 else nc.sync
                qa.dma_start(out=wq_st[:, fc, :], in_=wq[fc * 128:(fc + 1) * 128, :])
                qb.dma_start(out=wk_st[:, fc, :], in_=wk[fc * 128:(fc + 1) * 128, :])
            # Only lidar slab 0 loads up-front (behind the Wq chunks on
            # sync); slabs 1-3 are deferred one Q~-slab ahead so they
            # don't steal prologue bandwidth from cam^T. cam^T slabs
            # queue behind the Wk chunks on scalar.
            li_sts = {}

            def issue_li_dma(nb):
                li_st = stg.tile([128, DC, 512], F32, name=f"li_st_{nb}", tag="li_st")
                nc.sync.dma_start(out=li_st[:], in_=li_view[:, :, nb * 512:(nb + 1) * 512])
                li_sts[nb] = li_st

            issue_li_dma(0)
            # cam^T slabs split across BOTH queues in S(0)'s consumption
            # order (sync is idle after wq+li0; scalar carries wk) — the
            # last slab lands ~5us earlier than a single-queue stream
            cam_st = pa.tile([128, DC, M], F32)
            cam_view = cam_t[:, :].rearrange("(c p) m -> p c m", p=128)
            for mb, eng in ((0, nc.scalar), (1, nc.sync), (2, nc.scalar), (3, nc.sync)):
                sl = slice(mb * 512, (mb + 1) * 512)
                eng.dma_start(out=cam_st[:, :, sl], in_=cam_view[:, :, sl])

            pc_h = [
                psB.tile([128, 2, D], F32, name=f"pc_{h}", tag="Shalf", bufs=2)
                for h in range(2)
            ]
            for fc in range(DC):
                wq_r = wc.tile([128, D], F32R, name=f"wq_r_{fc}", tag="wq_r", bufs=2)
                wk_r = wc.tile([128, D], F32R, name=f"wk_r_{fc}", tag="wk_r", bufs=2)
                nc.vector.tensor_copy(wq_r[:], wq_st[:, fc, :])  # f32r rounding
                nc.vector.tensor_copy(wk_r[:], wk_st[:, fc, :])
                for gc in range(DC):
                    nc.tensor.matmul(
                        pc_h[gc // 2][:, gc % 2, :],
                        wq_r[:, gc * 128:(gc + 1) * 128],
                        wk_r[:],
                        start=(fc == 0),
                        stop=(fc == DC - 1),
                    )
            for gc in range(DC):
                # DVE, so ACT is free to cast cam slabs the moment they land
                nc.vector.tensor_copy(c_sb[:, gc, :], pc_h[gc // 2][:, gc % 2, :])

        # ---- main pipeline ----
        pexp = ctx.enter_context(tc.tile_pool(name="pexp", bufs=3))
        small = ctx.enter_context(tc.tile_pool(name="small", bufs=6))

        def emit_vproj(mbs):
            """V = cam Wv^T + bv (bf16) for the given cam slabs (the
            f32r casts for ALL slabs are emitted separately up front)."""
            for mb in mbs:
                for mt in range(4 * mb, 4 * mb + 4):
                    pv = psP.tile([128, D], F32, name=f"pv_{mt}", tag="proj")
                    for ec in range(DC):
                        nc.tensor.matmul(
                            pv[:],
                            cam_r[:, ec, mt * 128:(mt + 1) * 128],
                            wv_r[:, ec, :],
                            start=(ec == 0),
                            stop=(ec == DC - 1),
                        )
                    nc.vector.tensor_tensor(
                        out=v_sb[:, mt, :], in0=pv[:], in1=bv_bc[:], op=OP.add
                    )

        li_rs = {}

        def emit_qproj_group(nb, ec):
            """One e-chunk of Q~^T for column slab nb = (lidar C)^T +
            bias, *conf. ec==0 also casts the slab's lidar to f32r."""
            sl = slice(nb * 512, (nb + 1) * 512)
            if ec == 0:
                li_rs[nb] = stg.tile(
                    [128, DC, 512], F32R, name=f"li_r_{nb}", tag="li_r"
                )
                nc.vector.tensor_copy(li_rs[nb][:], li_sts[nb][:])  # f32r round
            pq = psP.tile([128, 512], F32, name=f"pq_{ec}_{nb}", tag="proj")
            for gc in range(DC):
                nc.tensor.matmul(
                    pq[:],
                    c_sb[:, gc, ec * 128:(ec + 1) * 128],
                    li_rs[nb][:, gc, :],
                    start=(gc == 0),
                    stop=(gc == DC - 1),
                )
            # q~^T = (proj + (bq Wk)[e]) * conf[n]
            nc.vector.scalar_tensor_tensor(
                out=qt[:, ec, sl],
                in0=pq[:],
                scalar=bq_t[:, ec:ec + 1],
                in1=conf_bc[:, sl],
                op0=OP.add,
                op1=OP.mult,
            )

        def emit_qproj(nb):
            if nb + 1 < NB:
                issue_li_dma(nb + 1)  # prefetch the next slab
            for ec in range(DC):
                emit_qproj_group(nb, ec)

        def emit_scores_softmax(t, mid=None):
            """S(t) in two half-width PSUM tiles (tag bufs=2, so tile t+1's
            first half only waits exp(t)'s first half); DVE partial
            row-max chases each 512-slab; exp in two ACT passes; returns
            (p_bf, sums). `mid` is emitted before the last cam slab's
            chunks (fills the PE if that slab's DMA is still in flight)."""
            halves = [
                psB.tile([128, M // 2], F32, name=f"s_{t}_{h}", tag="Shalf", bufs=2)
                for h in range(2)
            ]
            # Tapered slabs: the final 384/128 chunks shrink the last
            # partial row-max, shortening the S->exp critical chain.
            chunks = [(0, 512), (512, 1024), (1024, 1536), (1536, 1920), (1920, 2048)]
            pm = small.tile([128, len(chunks)], F32, name=f"pm_{t}", tag="pmax")
            for ci, (lo, hi) in enumerate(chunks):
                if ci == 3 and mid is not None:
                    mid()
                s_ps = halves[lo // 1024]
                sl = slice(lo % 1024, (hi - 1) % 1024 + 1)
                for ec in range(DC):
                    nc.tensor.matmul(
                        s_ps[:, sl],
                        qt[:, ec, t * 128:(t + 1) * 128],
                        cam_r[:, ec, lo:hi],
                        start=(ec == 0),
                        stop=(ec == DC - 1),
                    )
                nc.vector.tensor_reduce(
                    out=pm[:, ci:ci + 1], in_=s_ps[:, sl], axis=AX.X, op=OP.max
                )
            negmax = small.tile([128, 1], F32, name=f"nm_{t}", tag="negmax")
            nc.vector.tensor_reduce(
                out=negmax[:], in_=pm[:], axis=AX.X, op=OP.max, negate=True
            )
            p_bf = pexp.tile([128, M], BF16, name=f"p_{t}", tag="P")
            sums2 = small.tile([128, 2], F32, name=f"sum2_{t}", tag="sums2")
            for h in range(2):
                nc.scalar.activation(
                    out=p_bf[:, h * (M // 2):(h + 1) * (M // 2)],
                    in_=halves[h][:],
                    func=AF.Exp,
                    bias=negmax[:],
                    scale=1.0,
                    accum_out=sums2[:, h:h + 1],
                )
            sums = small.tile([128, 1], F32, name=f"sum_{t}", tag="sums")
            nc.vector.tensor_reduce(
                out=sums[:], in_=sums2[:], axis=AX.X, op=OP.add
            )
            return p_bf, sums

        def emit_pv(t, p_bf, sums):
            """P^T(t) on the PE (exp(t) long done) -> PV(t) -> out."""
            ptr = pexp.tile([128, MT, 128], BF16, name=f"ptr_{t}", tag="PT")
            for g in range(MT // 8):
                ptp = psT.tile([128, 8, 128], BF16, name=f"ptp_{t}_{g}", tag="ptp")
                for c in range(8):
                    j = g * 8 + c
                    nc.tensor.transpose(
                        ptp[:, c, :], p_bf[:, j * 128:(j + 1) * 128], ident[:]
                    )
                # DVE (not ACT) so exp(t+1) is never queued behind these
                nc.vector.tensor_copy(ptr[:, g * 8:(g + 1) * 8, :], ptp[:])

            o_ps = psP.tile([128, D], F32, name=f"o_{t}", tag="proj")
            for j in range(MT):
                nc.tensor.matmul(
                    o_ps[:],
                    ptr[:, j, :],
                    v_sb[:, j, :],
                    start=(j == 0),
                    stop=(j == MT - 1),
                )
            recip = small.tile([128, 1], F32, name=f"rc_{t}", tag="recip")
            nc.vector.reciprocal(recip[:], sums[:])
            o_sb = pexp.tile([128, D], F32, name=f"o_sb_{t}", tag="Osb")
            nc.vector.tensor_scalar_mul(out=o_sb[:], in0=o_ps[:], scalar1=recip[:])
            # sync HW queue — idle after phase A; the gpsimd software
            # queue adds ~10us of dispatch latency to the final tile
            nc.sync.dma_start(out=out[t * 128:(t + 1) * 128, :], in_=o_sb[:])

        # cam^T f32r casts (ACT) as the slabs land — S(0) and V both
        # consume cam_r
        for mb in range(MB):
            sl = slice(mb * 512, (mb + 1) * 512)
            nc.scalar.copy(cam_r[:, :, sl], cam_st[:, :, sl])

        # PE order: C -> Q~0 -> S(0) as soon as the last cam cast lands
        # (V is NOT on S(0)'s critical path — it slots into the tile-0/1
        # softmax windows instead; PV(0) waits until V completes). Then
        # steady state: Q~ slab nb lands right before the 4 tiles that
        # read it, and PV(t-1) + Q~ inserts cover softmax(t) latency.
        emit_qproj(0)
        pending = None
        for t in range(NT):
            if t % 4 == 0 and t > 0:
                emit_qproj(t // 4)
            cur = emit_scores_softmax(t)
            if t == 0:
                emit_vproj([0, 1])
            elif t == 1:
                emit_vproj([2, 3])
                emit_pv(0, *pending)
            elif pending is not None:
                emit_pv(t - 1, *pending)
            pending = cur
        emit_pv(NT - 1, *pending)

    nc.compile()
    return nc


_NC_CACHE = None


def make_in_maps(inputs) -> list[dict]:
    def f32(name):
        return np.ascontiguousarray(np.asarray(inputs[name]), dtype=np.float32)

    li, ca, lc = f32("lidar_features"), f32("camera_features"), f32("lidar_confidence")
    wq_, wk_ = f32("Wq"), f32("Wk")
    wvt_ = np.ascontiguousarray(f32("Wv").T)
    bqk_ = np.ascontiguousarray(f32("bq") @ wk_)  # (bq Wk)[e]
    bv_ = f32("bv")

    return [
        {
            "lidar_t": np.ascontiguousarray(li[b].T),
            "cam_t": np.ascontiguousarray(ca[b].T),
            "lconf": lc[b],
            "wq": wq_, "wk": wk_, "wvt": wvt_,
            "bqk": bqk_, "bv": bv_,
        }
        for b in range(B)
    ]


def kernel(**inputs) -> np.ndarray:
    global _NC_CACHE
    if _NC_CACHE is None:
        _NC_CACHE = build()
    nc = _NC_CACHE

    res = run_bass_kernel_spmd(nc, make_in_maps(inputs), list(range(B)))
    return np.stack([res.results[b]["out"] for b in range(B)]).astype(np.float32)
